# revision 16
# baseline (speedup 1.0000x reference)
"""Trainium2 Bass kernel for nn_Attention_25159918420763 (distillation attention loss).

Strategy (8 NeuronCores, data-parallel over batch: 64 items -> 8 per core):
  Stage A (memory-bound, ~60MB/core): stream every feature map through SBUF once,
    computing two reductions per level:
      - channel sums  (DVE free-axis reduce)            -> feed key/query matmuls
      - spatial sum-of-squares (ACT square + PE ones-matmul over channel partitions)
        -> attention "value" maps
  Stage B (tiny): pooled-map cascade + L2 norms + cross dots (DVE),
    key/query matmuls + BatchNorm with cross-core stats (AllReduce #1),
    bilinear matmul + BatchNorm (AllReduce #2), logits + softmax + weighted
    diff loss, final loss mean (AllReduce #3).

All BN/pooling scale factors are folded or cancel:
  - chan mean = chan sum / hw is folded into host-prescaled Wq/Wk.
  - pooled maps are block *sums*; L2 normalization cancels the scale.
"""

import os

import numpy as np

BS = 64
NCORES = 8
BLOC = BS // NCORES  # 8 local batch items
QK = 128
BN_EPS = 1e-5
S_SHAPES = [(64, 56, 56), (128, 28, 28), (256, 14, 14), (512, 7, 7)]
T_SHAPES = [(256, 56, 56), (512, 28, 28), (1024, 14, 14), (2048, 7, 7)]
HWS = [3136, 784, 196, 49]
RES = [56, 28, 14, 7]


def _chunks(hw):
    # spatial chunks <= 512 for one PSUM bank, uniform
    if hw == 3136:
        return [448] * 7
    if hw == 784:
        return [392] * 2
    return [hw]


def build(nc):
    import concourse.bass as bass
    import concourse.tile as tile
    from concourse import mybir

    f32 = mybir.dt.float32
    AF = mybir.ActivationFunctionType
    ALU = mybir.AluOpType
    RG = [list(range(NCORES))]

    # ---------------- DRAM parameters ----------------
    def P(name, shape):
        return nc.declare_dram_parameter(name, list(shape), f32, isOutput=False)

    g_s = [P(f"g_s{j}", (BLOC, c, h, w)) for j, (c, h, w) in enumerate(S_SHAPES)]
    g_t = [P(f"g_t{i}", (BLOC, c, h, w)) for i, (c, h, w) in enumerate(T_SHAPES)]
    # weights: prescaled by 1/hw on host; [c, 128]
    wk = [P(f"wk{j}", (S_SHAPES[j][0], QK)) for j in range(4)]
    wq = [P(f"wq{i}", (T_SHAPES[i][0], QK)) for i in range(4)]
    wb = P("wb", (QK, 512))
    # transposed small params: layers 0-3 = q0..q3, 4-7 = k0..k3
    bT8 = P("bT8", (QK, 8))
    gT8 = P("gT8", (QK, 8))
    betaT8 = P("betaT8", (QK, 8))
    bbT = P("bbT", (QK, 4))
    gbT = P("gbT", (QK, 4))
    betabT = P("betabT", (QK, 4))
    mask8 = P("mask8", (BLOC, BLOC * 4))
    out_ext = nc.declare_dram_parameter("out", [1, 4], f32, isOutput=True)

    with tile.TileContext(nc) as tc:
        with (
            tc.tile_pool(name="xin", bufs=6) as xin_pool,
            tc.tile_pool(name="xsq", bufs=4) as xsq_pool,
            tc.tile_pool(name="persist", bufs=1) as pp,
            tc.tile_pool(name="small", bufs=2) as sp,
            tc.tile_pool(name="pmap", bufs=4, space="PSUM") as pmap_pool,
            tc.tile_pool(name="pmm", bufs=2, space="PSUM") as pmm_pool,
            tc.tile_pool(name="dram", bufs=1, space="DRAM") as dram_pool,
        ):
            # ---------------- constants ----------------
            ones128 = pp.tile([128, 1], f32, tag="ones128")
            nc.vector.memset(ones128[:, :], 1.0)
            half2 = pp.tile([128, 2], f32, tag="half2")
            nc.vector.memset(half2[:, :], 0.0)
            nc.vector.memset(half2[0:64, 0:1], 1.0)
            nc.vector.memset(half2[64:128, 1:2], 1.0)
            # eye8[:, b, m] = 1 iff m == b  (lhsT selecting output row b)
            eye8 = pp.tile([128, 8, 8], f32, tag="eye8")
            nc.vector.memset(eye8[:, :, :], 0.0)
            for b in range(8):
                nc.vector.memset(eye8[:, b, b : b + 1], 1.0)
            # s0eye[:, tt, :]: col 2tt ones on partitions 0-63, col 2tt+1 on 64-127
            s0eye = pp.tile([128, 4, 8], f32, tag="s0eye")
            nc.vector.memset(s0eye[:, :, :], 0.0)
            for tt in range(4):
                nc.vector.memset(s0eye[0:64, tt, 2 * tt : 2 * tt + 1], 1.0)
                nc.vector.memset(s0eye[64:128, tt, 2 * tt + 1 : 2 * tt + 2], 1.0)
            epsT = pp.tile([128, 1], f32, tag="epsT")
            nc.vector.memset(epsT[:, :], BN_EPS)
            mask_sb = pp.tile([BLOC, BLOC * 4], f32, tag="mask_sb")
            nc.sync.dma_start(out=mask_sb[:, :], in_=mask8[:, :])

            # spatial sum-of-squares maps: one [8, hw] tile per map so every
            # compute op sees partition-0-aligned operands.
            # native maps mt[(kind, lv)]; pooled-down pdt[(kind, lv, ridx)]
            mt = {}
            for kind in ("s", "t"):
                for lv in range(4):
                    mt[(kind, lv)] = pp.tile(
                        [8, HWS[lv]], f32,
                        tag=f"map_{kind}{lv}", name=f"map_{kind}{lv}",
                    )
            pdt = {}
            for kind in ("s", "t"):
                for lv in range(4):
                    for ridx in range(lv + 1, 4):
                        pdt[(kind, lv, ridx)] = pp.tile(
                            [8, HWS[ridx]], f32,
                            tag=f"pd_{kind}{lv}_{ridx}",
                            name=f"pd_{kind}{lv}_{ridx}",
                        )

            # channel-sum tiles per level: [128, n_ct, BLOC]
            # levels: ('s', j) and ('t', i)
            def n_ct(kind, lv):
                c = (S_SHAPES if kind == "s" else T_SHAPES)[lv][0]
                return max(1, c // 128)

            cs = {}
            for kind in ("s", "t"):
                for lv in range(4):
                    cs[(kind, lv)] = pp.tile(
                        [128, n_ct(kind, lv), BLOC], f32,
                        tag=f"cs_{kind}{lv}", name=f"cs_{kind}{lv}",
                    )
            # s0 raw (2 items interleaved per 128 partitions): [128, 4 pairs]
            cs0raw = pp.tile([128, 4], f32, tag="cs0raw")

            # ---------------- Stage A: streaming reductions ----------------
            # per level: (kind, lv, dram handle, c, hw)
            levels = []
            for lv in range(4):
                levels.append(("s", lv, g_s[lv], S_SHAPES[lv][0], HWS[lv]))
                levels.append(("t", lv, g_t[lv], T_SHAPES[lv][0], HWS[lv]))

            # partial chan sums for chunked levels: [128, n_ct, BLOC, n_chunk]
            partials = {}
            for kind, lv, gd, c, hw in levels:
                nch = len(_chunks(hw))
                if kind == "s" and lv == 0:
                    partials[(kind, lv)] = pp.tile(
                        [128, 4, nch], f32, tag="pt_s0", name="pt_s0"
                    )
                else:
                    partials[(kind, lv)] = pp.tile(
                        [128, n_ct(kind, lv), BLOC, nch], f32,
                        tag=f"pt_{kind}{lv}", name=f"pt_{kind}{lv}",
                    )

            for kind, lv, gd, c, hw in levels:
                cks = _chunks(hw)
                nch = len(cks)
                Lres = mt[(kind, lv)]
                flat = gd.ap().rearrange("b c h w -> (b c) (h w)")
                pt = partials[(kind, lv)]

                if kind == "s" and lv == 0:
                    # c=64: two items per 128-partition tile; 4 pairs
                    co = 0
                    for ich, ck in enumerate(cks):
                        psum = pmap_pool.tile([8, ck], f32, tag="psum_map")
                        for tt in range(4):
                            x = xin_pool.tile([128, ck], f32, tag="x")
                            nc.sync.dma_start(
                                out=x[:, :],
                                in_=flat[tt * 128 : (tt + 1) * 128, co : co + ck],
                            )
                            x2 = xsq_pool.tile([128, ck], f32, tag="x2")
                            nc.scalar.activation(x2[:, :], x[:, :], AF.Square)
                            nc.vector.reduce_sum(
                                pt[:, tt, ich : ich + 1], x[:, :], axis=mybir.AxisListType.X
                            )
                            nc.tensor.matmul(
                                psum[:, :],
                                s0eye[:, tt, :],
                                x2[:, :],
                                start=(tt == 0),
                                stop=(tt == 3),
                            )
                        nc.scalar.activation(
                            Lres[:, co : co + ck], psum[:, :], AF.Copy
                        )
                        co += ck
                else:
                    nct = n_ct(kind, lv)
                    if hw == 49:
                        # batched DMA: whole item [c, 49] -> [128, nct, 49]
                        r3 = gd.ap().rearrange(
                            "b (ct p) h w -> b p ct (h w)", p=128
                        )
                        psum = pmap_pool.tile([8, 49], f32, tag="psum_map")
                        for b in range(BLOC):
                            x = xin_pool.tile([128, nct, 49], f32, tag="x")
                            nc.sync.dma_start(out=x[:, :, :], in_=r3[b])
                            x2 = xsq_pool.tile([128, nct, 49], f32, tag="x2")
                            nc.scalar.activation(x2[:, :, :], x[:, :, :], AF.Square)
                            nc.vector.reduce_sum(
                                pt[:, :, b, 0], x[:, :, :], axis=mybir.AxisListType.X
                            )
                            for ct in range(nct):
                                nc.tensor.matmul(
                                    psum[:, :],
                                    eye8[:, b, :],
                                    x2[:, ct, :],
                                    start=(b == 0 and ct == 0),
                                    stop=(b == BLOC - 1 and ct == nct - 1),
                                )
                        nc.scalar.activation(
                            Lres[:, :], psum[:, :], AF.Copy
                        )
                    else:
                        co = 0
                        for ich, ck in enumerate(cks):
                            psum = pmap_pool.tile([8, ck], f32, tag="psum_map")
                            for b in range(BLOC):
                                for ct in range(nct):
                                    x = xin_pool.tile([128, ck], f32, tag="x")
                                    r0 = b * c + ct * 128
                                    nc.sync.dma_start(
                                        out=x[:, :],
                                        in_=flat[r0 : r0 + 128, co : co + ck],
                                    )
                                    x2 = xsq_pool.tile([128, ck], f32, tag="x2")
                                    nc.scalar.activation(x2[:, :], x[:, :], AF.Square)
                                    nc.vector.reduce_sum(
                                        pt[:, ct, b, ich : ich + 1],
                                        x[:, :],
                                        axis=mybir.AxisListType.X,
                                    )
                                    nc.tensor.matmul(
                                        psum[:, :],
                                        eye8[:, b, :],
                                        x2[:, :],
                                        start=(b == 0 and ct == 0),
                                        stop=(b == BLOC - 1 and ct == nct - 1),
                                    )
                            nc.scalar.activation(
                                Lres[:, co : co + ck],
                                psum[:, :],
                                AF.Copy,
                            )
                            co += ck

            stage = os.environ.get("KSTAGE", "full")

            def bail():
                z = sp.tile([1, 4], f32, tag="bailz", name="bailz")
                nc.vector.memset(z[:, :], 0.0)
                nc.sync.dma_start(out=out_ext[:, :], in_=z[:, :])

            if stage == "A0":
                bail()
                return nc

            # ---------------- finalize channel sums ----------------
            for kind, lv, gd, c, hw in levels:
                pt = partials[(kind, lv)]
                if kind == "s" and lv == 0:
                    nc.vector.reduce_sum(
                        cs0raw[:, :], pt[:, :, :], axis=mybir.AxisListType.X
                    )
                    # de-interleave: even items at partitions 0-63, odd at 64-127
                    csA = cs[("s", 0)]  # [128, 1, 8] (only partitions 0-63 used)
                    csA_v = csA[0:64, 0, :].rearrange("p (b two) -> p two b", two=2)
                    nc.sync.dma_start(out=csA_v[:, 0, :], in_=cs0raw[0:64, :])
                    nc.sync.dma_start(out=csA_v[:, 1, :], in_=cs0raw[64:128, :])
                else:
                    nc.vector.reduce_sum(
                        cs[(kind, lv)][:, :, :], pt[:, :, :, :], axis=mybir.AxisListType.X
                    )

            if stage == "A":
                bail()
                return nc

            # ---------------- pooled-map cascade (block sums) ----------------
            def pool_step(src_t, dst_t, h, w):
                """src [8, h*w] -> dst [8, (h/2)*(w/2)], 2x2 block sums."""
                w2, h2 = w // 2, h // 2
                tmp = sp.tile([8, h * w2], f32, tag="pooltmp", name="pooltmp")
                nc.vector.reduce_sum(
                    tmp[:, :],
                    src_t.rearrange(
                        "p (h w2 two) -> p (h w2) two", h=h, w2=w2, two=2
                    ),
                    axis=mybir.AxisListType.X,
                )
                nc.vector.reduce_sum(
                    dst_t,
                    tmp.rearrange(
                        "p (h2 hp w2) -> p h2 w2 hp", h2=h2, hp=2, w2=w2
                    ),
                    axis=mybir.AxisListType.X,
                )

            for kind in ("s", "t"):
                for lv in range(4):
                    prev = mt[(kind, lv)][:, :]
                    for ridx in range(lv + 1, 4):
                        dst = pdt[(kind, lv, ridx)]
                        pool_step(prev, dst[:, :], RES[ridx - 1], RES[ridx - 1])
                        prev = dst[:, :]

            if stage == "B1":
                bail()
                return nc

            # ---------------- norms (1/L2) ----------------
            # needed: all natives; pooled student maps (for j finer than i)
            rn = {}

            def calc_rn(key_, ap, hw, tagn):
                prod = sp.tile([8, hw], f32, tag="normprod", name="normprod")
                n2 = sp.tile([8, 1], f32, tag="n2", name="n2")
                nc.scalar.activation(prod[:, :], ap, AF.Square)
                nc.vector.reduce_sum(n2[:, :], prod[:, :], axis=mybir.AxisListType.X)
                sd = sp.tile([8, 1], f32, tag="normsd", name="normsd")
                nc.scalar.activation(sd[:, :], n2[:, :], AF.Sqrt)
                r = pp.tile([8, 1], f32, tag=f"rn_{tagn}", name=f"rn_{tagn}")
                nc.vector.reciprocal(r[:, :], sd[:, :])
                rn[key_] = r

            for kind in ("s", "t"):
                for lv in range(4):
                    calc_rn((kind, lv, lv), mt[(kind, lv)][:, :], HWS[lv],
                            f"{kind}{lv}n")
            for lv in range(4):
                for ridx in range(lv + 1, 4):
                    calc_rn(("s", lv, ridx), pdt[("s", lv, ridx)][:, :],
                            HWS[ridx], f"s{lv}p{ridx}")

            if stage == "B2":
                bail()
                return nc

            # ---------------- dots + diff ----------------
            # D[b, i*4+j] = mean_p (value_s[i][b,j] - value_t[i][b])^2
            D = pp.tile([BLOC, 16], f32, tag="D")
            for ti in range(4):
                for sj in range(4):
                    if sj <= ti:
                        # student at res sj is >= teacher res ti (finer or equal):
                        # pool student down to res ti (or native if equal)
                        u = (mt[("s", sj)] if sj == ti else pdt[("s", sj, ti)])[:, :]
                        v = mt[("t", ti)][:, :]
                        un = rn[("s", sj, ti)]
                        hwd = HWS[ti]
                        r2 = 1.0
                    else:
                        # student coarser: dot(S_j, blocksum(T_i -> res sj))
                        u = mt[("s", sj)][:, :]
                        v = pdt[("t", ti, sj)][:, :]
                        un = rn[("s", sj, sj)]
                        hwd = HWS[sj]
                        r2 = float(HWS[ti]) / HWS[sj]
                    vn = rn[("t", ti, ti)]
                    prod = sp.tile([8, hwd], f32, tag="dotprod", name="dotprod")
                    dot = sp.tile([8, 1], f32, tag="dot", name="dot")
                    nc.vector.tensor_mul(prod[:, :], u, v)
                    nc.vector.reduce_sum(dot[:, :], prod[:, :], axis=mybir.AxisListType.X)
                    nc.vector.tensor_mul(dot[:, :], dot[:, :], un[:, :])
                    nc.vector.tensor_mul(dot[:, :], dot[:, :], vn[:, :])
                    hw_i = HWS[ti]
                    a = -2.0 / (hw_i * float(np.sqrt(r2)))
                    col = ti * 4 + sj
                    nc.vector.tensor_scalar(
                        out=D[:, col : col + 1],
                        in0=dot[:, :],
                        scalar1=a,
                        scalar2=2.0 / hw_i,
                        op0=ALU.mult,
                        op1=ALU.add,
                    )

            if stage == "B":
                bail()
                return nc

            # ---------------- key/query matmuls + BN stats ----------------
            # weights to SBUF: [128, n_ct, 128]
            w_sb = {}
            for kind in ("s", "t"):
                for lv in range(4):
                    c = (S_SHAPES if kind == "s" else T_SHAPES)[lv][0]
                    wd = (wk if kind == "s" else wq)[lv]
                    if c >= 128:
                        nct = c // 128
                        t = pp.tile(
                            [128, nct, QK], f32,
                            tag=f"w_{kind}{lv}", name=f"w_{kind}{lv}",
                        )
                        nc.sync.dma_start(
                            out=t[:, :, :],
                            in_=wd.ap().rearrange("(ct p) q -> p ct q", p=128),
                        )
                    else:
                        t = pp.tile(
                            [64, 1, QK], f32,
                            tag=f"w_{kind}{lv}", name=f"w_{kind}{lv}",
                        )
                        nc.sync.dma_start(out=t[:, 0, :], in_=wd.ap())
                    w_sb[(kind, lv)] = t

            bT_sb = pp.tile([128, 8], f32, tag="bT_sb")
            nc.sync.dma_start(out=bT_sb[:, :], in_=bT8.ap())
            gT_sb = pp.tile([128, 8], f32, tag="gT_sb")
            nc.sync.dma_start(out=gT_sb[:, :], in_=gT8.ap())
            betaT_sb = pp.tile([128, 8], f32, tag="betaT_sb")
            nc.sync.dma_start(out=betaT_sb[:, :], in_=betaT8.ap())

            # layer order: 0-3 q (teacher levels), 4-7 k (student levels)
            LAYERS = [("t", i) for i in range(4)] + [("s", j) for j in range(4)]
            yT = pp.tile([128, 8, BLOC], f32, tag="yT")  # per layer [128, 8]
            ST = pp.tile([128, 8, 2], f32, tag="ST")
            for l, (kind, lv) in enumerate(LAYERS):
                w = w_sb[(kind, lv)]
                x = cs[(kind, lv)]
                kp = w.shape[0]  # 64 for s0 else 128
                nct = w.shape[1]
                py = pmm_pool.tile([128, BLOC], f32, tag="pmm")
                for ct in range(nct):
                    nc.tensor.matmul(
                        py[:, :],
                        w[0:kp, ct, :],
                        x[0:kp, ct, :],
                        start=(ct == 0),
                        stop=(ct == nct - 1),
                    )
                nc.scalar.activation(
                    yT[:, l, :], py[:, :], AF.Identity, bias=bT_sb[:, l : l + 1]
                )
                nc.vector.reduce_sum(
                    ST[:, l, 0:1], yT[:, l, :], axis=mybir.AxisListType.X
                )
                ysq = sp.tile([128, BLOC], f32, tag="ysq")
                nc.scalar.activation(ysq[:, :], yT[:, l, :], AF.Square)
                nc.vector.reduce_sum(
                    ST[:, l, 1:2], ysq[:, :], axis=mybir.AxisListType.X
                )

            # ---------------- AllReduce #1 (BN stats for q/k) ----------------
            cc1_in = dram_pool.tile([128, 16], f32)
            cc1_out = dram_pool.tile([128, 16], f32)
            nc.sync.dma_start(out=cc1_in[:, :], in_=ST[:, :, :])
            nc.gpsimd.collective_compute(
                "AllReduce",
                ALU.add,
                ins=[cc1_in.opt()],
                outs=[cc1_out.opt()],
                replica_groups=RG,
            )
            STg = pp.tile([128, 8, 2], f32, tag="STg")
            nc.sync.dma_start(out=STg[:, :, :], in_=cc1_out[:, :])

            # BN params vectorized over 8 layers
            def bn_params(STg_ap, nlayers, count, g_ap, beta_ap, tagp):
                m = sp.tile([128, nlayers], f32, tag=f"m{tagp}")
                nc.vector.tensor_scalar(
                    out=m[:, :], in0=STg_ap[:, :, 0], scalar1=1.0 / count,
                    scalar2=None, op0=ALU.mult,
                )
                ex2 = sp.tile([128, nlayers], f32, tag=f"ex2{tagp}")
                nc.vector.tensor_scalar(
                    out=ex2[:, :], in0=STg_ap[:, :, 1], scalar1=1.0 / count,
                    scalar2=None, op0=ALU.mult,
                )
                m2 = sp.tile([128, nlayers], f32, tag=f"m2{tagp}")
                nc.scalar.activation(m2[:, :], m[:, :], AF.Square)
                v = sp.tile([128, nlayers], f32, tag=f"v{tagp}")
                nc.vector.tensor_sub(v[:, :], ex2[:, :], m2[:, :])
                sd = sp.tile([128, nlayers], f32, tag=f"sd{tagp}")
                nc.scalar.activation(
                    sd[:, :], v[:, :], AF.Sqrt, bias=epsT[:, :]
                )
                rstd = sp.tile([128, nlayers], f32, tag=f"rstd{tagp}")
                nc.vector.reciprocal(rstd[:, :], sd[:, :])
                sc = pp.tile([128, nlayers], f32, tag=f"sc{tagp}")
                nc.vector.tensor_mul(sc[:, :], g_ap, rstd[:, :])
                msc = sp.tile([128, nlayers], f32, tag=f"msc{tagp}")
                nc.vector.tensor_mul(msc[:, :], m[:, :], sc[:, :])
                sh = pp.tile([128, nlayers], f32, tag=f"sh{tagp}")
                nc.vector.tensor_sub(sh[:, :], beta_ap, msc[:, :])
                return sc, sh

            sc8, sh8 = bn_params(STg, 8, float(BS), gT_sb[:, :], betaT_sb[:, :], "a")

            # apply: qn (layers 0-3), kn (layers 4-7, +relu)
            qn = pp.tile([128, 4, BLOC], f32, tag="qn")
            kn = pp.tile([128, 4, BLOC], f32, tag="kn")
            for l in range(8):
                dst = qn[:, l, :] if l < 4 else kn[:, l - 4, :]
                nc.vector.tensor_scalar(
                    out=dst,
                    in0=yT[:, l, :],
                    scalar1=sc8[:, l : l + 1],
                    scalar2=sh8[:, l : l + 1],
                    op0=ALU.mult,
                    op1=ALU.add,
                )
            for j in range(4):
                nc.scalar.activation(kn[:, j, :], kn[:, j, :], AF.Relu)

            # ---------------- bilinear ----------------
            wb_sb = pp.tile([128, 512], f32, tag="wb_sb")
            nc.sync.dma_start(out=wb_sb[:, :], in_=wb.ap())
            bbT_sb = pp.tile([128, 4], f32, tag="bbT_sb")
            nc.sync.dma_start(out=bbT_sb[:, :], in_=bbT.ap())
            gbT_sb = pp.tile([128, 4], f32, tag="gbT_sb")
            nc.sync.dma_start(out=gbT_sb[:, :], in_=gbT.ap())
            betabT_sb = pp.tile([128, 4], f32, tag="betabT_sb")
            nc.sync.dma_start(out=betabT_sb[:, :], in_=betabT.ap())

            # K32[:, b*4+j] = kn[:, j, b]
            K32 = pp.tile([128, 32], f32, tag="K32")
            for j in range(4):
                nc.scalar.activation(
                    K32.rearrange("p (b j) -> p b j", j=4)[:, :, j],
                    kn[:, j, :],
                    AF.Copy,
                )

            kbT = pp.tile([128, 4, 32], f32, tag="kbT")  # per t-block [128, 32]
            STb = pp.tile([128, 4, 2], f32, tag="STb")
            for m in range(4):
                pkb = pmm_pool.tile([128, 32], f32, tag="pmm")
                nc.tensor.matmul(
                    pkb[:, :],
                    wb_sb[:, m * 128 : (m + 1) * 128],
                    K32[:, :],
                    start=True,
                    stop=True,
                )
                nc.scalar.activation(
                    kbT[:, m, :], pkb[:, :], AF.Identity, bias=bbT_sb[:, m : m + 1]
                )
                nc.vector.reduce_sum(
                    STb[:, m, 0:1], kbT[:, m, :], axis=mybir.AxisListType.X
                )
                kbsq = sp.tile([128, 32], f32, tag="kbsq")
                nc.scalar.activation(kbsq[:, :], kbT[:, m, :], AF.Square)
                nc.vector.reduce_sum(
                    STb[:, m, 1:2], kbsq[:, :], axis=mybir.AxisListType.X
                )

            if stage == "C":
                bail()
                return nc

            # ---------------- AllReduce #2 (bilinear BN stats) ----------------
            cc2_in = dram_pool.tile([128, 8], f32)
            cc2_out = dram_pool.tile([128, 8], f32)
            nc.sync.dma_start(out=cc2_in[:, :], in_=STb[:, :, :])
            nc.gpsimd.collective_compute(
                "AllReduce",
                ALU.add,
                ins=[cc2_in.opt()],
                outs=[cc2_out.opt()],
                replica_groups=RG,
            )
            STbg = pp.tile([128, 4, 2], f32, tag="STbg")
            nc.sync.dma_start(out=STbg[:, :, :], in_=cc2_out[:, :])

            scb, shb = bn_params(
                STbg, 4, float(BS * 4), gbT_sb[:, :], betabT_sb[:, :], "b"
            )
            kbn = pp.tile([128, 4, 32], f32, tag="kbn")
            for m in range(4):
                nc.vector.tensor_scalar(
                    out=kbn[:, m, :],
                    in0=kbT[:, m, :],
                    scalar1=scb[:, m : m + 1],
                    scalar2=shb[:, m : m + 1],
                    op0=ALU.mult,
                    op1=ALU.add,
                )

            # ---------------- logits ----------------
            # full cross matmul per teacher i: out[b', (b,j)] then mask+reduce
            LTl = pp.tile([BLOC, 16], f32, tag="LTl")
            for i in range(4):
                pl = pmm_pool.tile([BLOC, 32], f32, tag="pmm")
                nc.tensor.matmul(
                    pl[:, :], qn[:, i, :], kbn[:, i, :], start=True, stop=True
                )
                ml = sp.tile([BLOC, 32], f32, tag="ml")
                nc.vector.tensor_mul(ml[:, :], pl[:, :], mask_sb[:, :])
                nc.vector.reduce_sum(
                    LTl[:, i * 4 : (i + 1) * 4],
                    ml.rearrange("p (b j) -> p j b", j=4),
                    axis=mybir.AxisListType.X,
                )

            if stage == "D":
                bail()
                return nc

            # ---------------- softmax + loss ----------------
            mx = sp.tile([BLOC, 4], f32, tag="mx")
            nc.vector.reduce_max(
                mx[:, :],
                LTl.rearrange("p (i j) -> p i j", j=4),
                axis=mybir.AxisListType.X,
            )
            LS = sp.tile([BLOC, 16], f32, tag="LS")
            for j in range(4):
                nc.vector.tensor_sub(
                    LS.rearrange("p (i j) -> p i j", j=4)[:, :, j],
                    LTl.rearrange("p (i j) -> p i j", j=4)[:, :, j],
                    mx[:, :],
                )
            E = sp.tile([BLOC, 16], f32, tag="E")
            nc.scalar.activation(E[:, :], LS[:, :], AF.Exp)
            Z = sp.tile([BLOC, 4], f32, tag="Z")
            nc.vector.reduce_sum(
                Z[:, :], E.rearrange("p (i j) -> p i j", j=4),
                axis=mybir.AxisListType.X,
            )
            ED = sp.tile([BLOC, 16], f32, tag="ED")
            nc.vector.tensor_mul(ED[:, :], E[:, :], D[:, :])
            NUM = sp.tile([BLOC, 4], f32, tag="NUM")
            nc.vector.reduce_sum(
                NUM[:, :], ED.rearrange("p (i j) -> p i j", j=4),
                axis=mybir.AxisListType.X,
            )
            Zi = sp.tile([BLOC, 4], f32, tag="Zi")
            nc.vector.reciprocal(Zi[:, :], Z[:, :])
            R8 = sp.tile([BLOC, 4], f32, tag="R8")
            nc.vector.tensor_mul(R8[:, :], NUM[:, :], Zi[:, :])
            ploss = pmm_pool.tile([1, 4], f32, tag="pmm")
            nc.tensor.matmul(
                ploss[:, :], ones128[0:BLOC, :], R8[:, :], start=True, stop=True
            )
            lossloc = sp.tile([1, 4], f32, tag="lossloc")
            nc.scalar.activation(
                lossloc[:, :], ploss[:, :], AF.Copy, scale=1.0 / BS
            )

            # ---------------- AllReduce #3 (loss mean) ----------------
            cc3_in = dram_pool.tile([1, 4], f32)
            cc3_out = dram_pool.tile([1, 4], f32)
            nc.sync.dma_start(out=cc3_in[:, :], in_=lossloc[:, :])
            nc.gpsimd.collective_compute(
                "AllReduce",
                ALU.add,
                ins=[cc3_in.opt()],
                outs=[cc3_out.opt()],
                replica_groups=RG,
            )
            nc.sync.dma_start(out=out_ext[:, :], in_=cc3_out[:, :])

    return nc


_CACHE = {}


def _build_and_finalize():
    if "nc" in _CACHE:
        return _CACHE["nc"]
    import concourse.bacc as bacc

    nc = bacc.Bacc("TRN2", target_bir_lowering=False, debug=False, num_devices=NCORES)
    build(nc)
    nc.finalize()
    _CACHE["nc"] = nc
    return nc


def _host_prep(inputs):
    """Build the 8 per-core input maps from full inputs."""
    a = {k: np.ascontiguousarray(np.asarray(v, dtype=np.float32)) for k, v in inputs.items()}
    shared = {}
    for j in range(4):
        shared[f"wk{j}"] = np.ascontiguousarray(a[f"Wk{j}"] / HWS[j])
    for i in range(4):
        shared[f"wq{i}"] = np.ascontiguousarray(a[f"Wq{i}"] / HWS[i])
    shared["wb"] = a["Wb"]
    # layers 0-3 = q, 4-7 = k; transposed to [128, 8]
    shared["bT8"] = np.ascontiguousarray(
        np.concatenate([a["bq"].T, a["bk"].T], axis=1)
    )
    shared["gT8"] = np.ascontiguousarray(
        np.concatenate([a["gq"].T, a["gk"].T], axis=1)
    )
    shared["betaT8"] = np.ascontiguousarray(
        np.concatenate([a["betaq"].T, a["betak"].T], axis=1)
    )
    shared["bbT"] = np.ascontiguousarray(a["bb"].reshape(4, 128).T)
    shared["gbT"] = np.ascontiguousarray(a["gb"].reshape(4, 128).T)
    shared["betabT"] = np.ascontiguousarray(a["betab"].reshape(4, 128).T)
    mask = np.zeros((BLOC, BLOC * 4), dtype=np.float32)
    for b in range(BLOC):
        mask[b, b * 4 : (b + 1) * 4] = 1.0
    shared["mask8"] = mask

    in_maps = []
    for cidx in range(NCORES):
        m = dict(shared)
        sl = slice(cidx * BLOC, (cidx + 1) * BLOC)
        for j in range(4):
            m[f"g_s{j}"] = np.ascontiguousarray(a[f"g_s{j}"][sl])
        for i in range(4):
            m[f"g_t{i}"] = np.ascontiguousarray(a[f"g_t{i}"][sl])
        in_maps.append(m)
    return in_maps


def run_on_hw(inputs, trace=False):
    from concourse.bass_utils import run_bass_kernel_spmd

    nc = _build_and_finalize()
    in_maps = _host_prep(inputs)
    res = run_bass_kernel_spmd(
        nc, in_maps, core_ids=list(range(NCORES)), trace=trace
    )
    return res


def kernel(**inputs):
    res = run_on_hw(inputs, trace=False)
    return np.asarray(res.results[0]["out"], dtype=np.float32).reshape(4)


# revision 18
# speedup vs baseline: 1.0855x; 1.0855x over previous
"""Trainium2 Bass kernel for nn_Attention_25159918420763 (distillation attention loss).

Strategy (8 NeuronCores, data-parallel over batch: 64 items -> 8 per core):
  Stage A (memory-bound, ~60MB/core): stream every feature map through SBUF once,
    computing two reductions per level:
      - channel sums  (DVE free-axis reduce)            -> feed key/query matmuls
      - spatial sum-of-squares (ACT square + PE ones-matmul over channel partitions)
        -> attention "value" maps
  Stage B (tiny): pooled-map cascade + L2 norms + cross dots (DVE),
    key/query matmuls + BatchNorm with cross-core stats (AllReduce #1),
    bilinear matmul + BatchNorm (AllReduce #2), logits + softmax + weighted
    diff loss, final loss mean (AllReduce #3).

All BN/pooling scale factors are folded or cancel:
  - chan mean = chan sum / hw is folded into host-prescaled Wq/Wk.
  - pooled maps are block *sums*; L2 normalization cancels the scale.
"""

import os

import numpy as np

BS = 64
NCORES = 8
BLOC = BS // NCORES  # 8 local batch items
QK = 128
BN_EPS = 1e-5
S_SHAPES = [(64, 56, 56), (128, 28, 28), (256, 14, 14), (512, 7, 7)]
T_SHAPES = [(256, 56, 56), (512, 28, 28), (1024, 14, 14), (2048, 7, 7)]
HWS = [3136, 784, 196, 49]
RES = [56, 28, 14, 7]


def _chunks(hw):
    # spatial chunks <= 512 for one PSUM bank, uniform
    if hw == 3136:
        return [448] * 7
    if hw == 784:
        return [392] * 2
    return [hw]


def build(nc):
    import concourse.bass as bass
    import concourse.tile as tile
    from concourse import mybir

    f32 = mybir.dt.float32
    AF = mybir.ActivationFunctionType
    ALU = mybir.AluOpType
    RG = [list(range(NCORES))]

    # ---------------- DRAM parameters ----------------
    def P(name, shape):
        return nc.declare_dram_parameter(name, list(shape), f32, isOutput=False)

    g_s = [P(f"g_s{j}", (BLOC, c, h, w)) for j, (c, h, w) in enumerate(S_SHAPES)]
    g_t = [P(f"g_t{i}", (BLOC, c, h, w)) for i, (c, h, w) in enumerate(T_SHAPES)]
    # weights: prescaled by 1/hw on host; [c, 128]
    wk = [P(f"wk{j}", (S_SHAPES[j][0], QK)) for j in range(4)]
    wq = [P(f"wq{i}", (T_SHAPES[i][0], QK)) for i in range(4)]
    wb = P("wb", (QK, 512))
    # transposed small params: layers 0-3 = q0..q3, 4-7 = k0..k3
    bT8 = P("bT8", (QK, 8))
    gT8 = P("gT8", (QK, 8))
    betaT8 = P("betaT8", (QK, 8))
    bbT = P("bbT", (QK, 4))
    gbT = P("gbT", (QK, 4))
    betabT = P("betabT", (QK, 4))
    mask8 = P("mask8", (BLOC, BLOC * 4))
    out_ext = nc.declare_dram_parameter("out", [1, 4], f32, isOutput=True)

    with tile.TileContext(nc) as tc:
        with (
            tc.tile_pool(name="xin", bufs=4) as xin_pool,
            tc.tile_pool(name="xsq", bufs=3) as xsq_pool,
            tc.tile_pool(name="persist", bufs=1) as pp,
            tc.tile_pool(name="small", bufs=2) as sp,
            tc.tile_pool(name="pmap", bufs=7, space="PSUM") as pmap_pool,
            tc.tile_pool(name="pmm", bufs=1, space="PSUM") as pmm_pool,
            tc.tile_pool(name="dram", bufs=1, space="DRAM") as dram_pool,
        ):
            # ---------------- constants ----------------
            ones128 = pp.tile([128, 1], f32, tag="ones128")
            nc.vector.memset(ones128[:, :], 1.0)
            half2 = pp.tile([128, 2], f32, tag="half2")
            nc.vector.memset(half2[:, :], 0.0)
            nc.vector.memset(half2[0:64, 0:1], 1.0)
            nc.vector.memset(half2[64:128, 1:2], 1.0)
            bf16 = mybir.dt.bfloat16
            # eye8[:, b, m] = 1 iff m == b  (lhsT selecting output row b)
            eye8 = pp.tile([128, 8, 8], bf16, tag="eye8")
            nc.vector.memset(eye8[:, :, :], 0.0)
            for b in range(8):
                nc.vector.memset(eye8[:, b, b : b + 1], 1.0)
            # s0eye[:, tt, :]: col 2tt ones on partitions 0-63, col 2tt+1 on 64-127
            s0eye = pp.tile([128, 4, 8], bf16, tag="s0eye")
            nc.vector.memset(s0eye[:, :, :], 0.0)
            for tt in range(4):
                nc.vector.memset(s0eye[0:64, tt, 2 * tt : 2 * tt + 1], 1.0)
                nc.vector.memset(s0eye[64:128, tt, 2 * tt + 1 : 2 * tt + 2], 1.0)
            epsT = pp.tile([128, 1], f32, tag="epsT")
            nc.vector.memset(epsT[:, :], BN_EPS)
            mask_sb = pp.tile([BLOC, BLOC * 4], f32, tag="mask_sb")
            nc.sync.dma_start(out=mask_sb[:, :], in_=mask8[:, :])

            # spatial sum-of-squares maps: one [8, hw] tile per map so every
            # compute op sees partition-0-aligned operands.
            # native maps mt[(kind, lv)]; pooled-down pdt[(kind, lv, ridx)]
            mt = {}
            for kind in ("s", "t"):
                for lv in range(4):
                    mt[(kind, lv)] = pp.tile(
                        [8, HWS[lv]], f32,
                        tag=f"map_{kind}{lv}", name=f"map_{kind}{lv}",
                    )
            pdt = {}
            for kind in ("s", "t"):
                for lv in range(4):
                    for ridx in range(lv + 1, 4):
                        pdt[(kind, lv, ridx)] = pp.tile(
                            [8, HWS[ridx]], f32,
                            tag=f"pd_{kind}{lv}_{ridx}",
                            name=f"pd_{kind}{lv}_{ridx}",
                        )

            # channel-sum tiles per level: [128, n_ct, BLOC]
            # levels: ('s', j) and ('t', i)
            def n_ct(kind, lv):
                c = (S_SHAPES if kind == "s" else T_SHAPES)[lv][0]
                return max(1, c // 128)

            cs = {}
            for kind in ("s", "t"):
                for lv in range(4):
                    cs[(kind, lv)] = pp.tile(
                        [128, n_ct(kind, lv), BLOC], f32,
                        tag=f"cs_{kind}{lv}", name=f"cs_{kind}{lv}",
                    )
            # s0 raw (2 items interleaved per 128 partitions): [128, 4 pairs]
            cs0raw = pp.tile([128, 4], f32, tag="cs0raw")

            # ---------------- Stage A: streaming reductions ----------------
            # Per (item, c-tile): ONE full-row DMA; ACT casts fp32->bf16 with
            # accum_out giving the channel sums for free; DVE squares in bf16;
            # PE ones-matmuls the squares into per-chunk PSUM map rows.
            levels = []
            for lv in range(4):
                levels.append(("t", lv, g_t[lv], T_SHAPES[lv][0], HWS[lv]))
                levels.append(("s", lv, g_s[lv], S_SHAPES[lv][0], HWS[lv]))

            for kind, lv, gd, c, hw in levels:
                cks = _chunks(hw)
                nch = len(cks)
                cos = [sum(cks[:i]) for i in range(nch)]
                Lres = mt[(kind, lv)]
                flat = gd.ap().rearrange("b c h w -> (b c) (h w)")
                nct = max(1, c // 128)
                psums = [
                    pmap_pool.tile(
                        [8, ck], f32, tag="psum_map", name=f"pm_{kind}{lv}_{i}"
                    )
                    for i, ck in enumerate(cks)
                ]

                if kind == "s" and lv == 0:
                    # c=64: two items interleaved per 128-partition tile
                    for tt in range(4):
                        x = xin_pool.tile([128, hw], f32, tag="x", name="x")
                        nc.sync.dma_start(
                            out=x[:, :], in_=flat[tt * 128 : (tt + 1) * 128, :]
                        )
                        xb = xin_pool.tile([128, hw], bf16, tag="xb", name="xb")
                        nc.scalar.activation(
                            xb[:, :], x[:, :], AF.Identity,
                            accum_out=cs0raw[:, tt : tt + 1],
                        )
                        x2b = xsq_pool.tile([128, hw], bf16, tag="x2b", name="x2b")
                        nc.vector.tensor_mul(x2b[:, :], xb[:, :], xb[:, :])
                        for ich, ck in enumerate(cks):
                            nc.tensor.matmul(
                                psums[ich][:, :],
                                s0eye[:, tt, :],
                                x2b[:, cos[ich] : cos[ich] + ck],
                                start=(tt == 0),
                                stop=(tt == 3),
                            )
                elif hw == 49:
                    # batch ALL c-tiles of one item into one DMA
                    for b in range(BLOC):
                        x = xin_pool.tile([128, nct, 49], f32, tag="x", name="x")
                        nc.sync.dma_start(
                            out=x[:, :, :],
                            in_=flat[b * c : (b + 1) * c, :].rearrange(
                                "(ct p) w -> p ct w", p=128
                            ),
                        )
                        xb = xin_pool.tile([128, nct, 49], bf16, tag="xb", name="xb")
                        nc.scalar.activation(xb[:, :, :], x[:, :, :], AF.Identity)
                        nc.vector.reduce_sum(
                            cs[(kind, lv)][:, :, b], xb[:, :, :],
                            axis=mybir.AxisListType.X,
                        )
                        x2b = xsq_pool.tile(
                            [128, nct, 49], bf16, tag="x2b", name="x2b"
                        )
                        nc.vector.tensor_mul(x2b[:, :, :], xb[:, :, :], xb[:, :, :])
                        for ct in range(nct):
                            nc.tensor.matmul(
                                psums[0][:, :],
                                eye8[:, b, :],
                                x2b[:, ct, :],
                                start=(b == 0 and ct == 0),
                                stop=(b == BLOC - 1 and ct == nct - 1),
                            )
                elif hw == 196:
                    # batch 2 c-tiles per DMA
                    ng = nct // 2
                    for b in range(BLOC):
                        for g in range(ng):
                            r0 = b * c + g * 256
                            x = xin_pool.tile([128, 2, 196], f32, tag="x", name="x")
                            nc.sync.dma_start(
                                out=x[:, :, :],
                                in_=flat[r0 : r0 + 256, :].rearrange(
                                    "(two p) w -> p two w", p=128
                                ),
                            )
                            xb = xin_pool.tile(
                                [128, 2, 196], bf16, tag="xb", name="xb"
                            )
                            for u in range(2):
                                nc.scalar.activation(
                                    xb[:, u, :], x[:, u, :], AF.Identity,
                                    accum_out=cs[(kind, lv)][
                                        :, 2 * g + u, b : b + 1
                                    ],
                                )
                            x2b = xsq_pool.tile(
                                [128, 2, 196], bf16, tag="x2b", name="x2b"
                            )
                            nc.vector.tensor_mul(
                                x2b[:, :, :], xb[:, :, :], xb[:, :, :]
                            )
                            for u in range(2):
                                ct = 2 * g + u
                                nc.tensor.matmul(
                                    psums[0][:, :],
                                    eye8[:, b, :],
                                    x2b[:, u, :],
                                    start=(b == 0 and ct == 0),
                                    stop=(b == BLOC - 1 and ct == nct - 1),
                                )
                else:
                    # hw in (3136, 784): one full c-tile per DMA
                    for b in range(BLOC):
                        for ct in range(nct):
                            r0 = b * c + ct * 128
                            x = xin_pool.tile([128, hw], f32, tag="x", name="x")
                            nc.sync.dma_start(
                                out=x[:, :], in_=flat[r0 : r0 + 128, :]
                            )
                            xb = xin_pool.tile([128, hw], bf16, tag="xb", name="xb")
                            nc.scalar.activation(
                                xb[:, :], x[:, :], AF.Identity,
                                accum_out=cs[(kind, lv)][:, ct, b : b + 1],
                            )
                            x2b = xsq_pool.tile(
                                [128, hw], bf16, tag="x2b", name="x2b"
                            )
                            nc.vector.tensor_mul(x2b[:, :], xb[:, :], xb[:, :])
                            for ich, ck in enumerate(cks):
                                nc.tensor.matmul(
                                    psums[ich][:, :],
                                    eye8[:, b, :],
                                    x2b[:, cos[ich] : cos[ich] + ck],
                                    start=(b == 0 and ct == 0),
                                    stop=(b == BLOC - 1 and ct == nct - 1),
                                )

                for ich, ck in enumerate(cks):
                    nc.scalar.activation(
                        Lres[:, cos[ich] : cos[ich] + ck], psums[ich][:, :], AF.Copy
                    )

            stage = os.environ.get("KSTAGE", "full")

            def bail():
                z = sp.tile([1, 4], f32, tag="bailz", name="bailz")
                nc.vector.memset(z[:, :], 0.0)
                nc.sync.dma_start(out=out_ext[:, :], in_=z[:, :])

            if stage == "A0":
                bail()
                return nc

            # ---------------- finalize s0 channel sums (de-interleave) ----------------
            csA = cs[("s", 0)]  # [128, 1, 8] (only partitions 0-63 used)
            csA_v = csA[0:64, 0, :].rearrange("p (b two) -> p two b", two=2)
            nc.sync.dma_start(out=csA_v[:, 0, :], in_=cs0raw[0:64, :])
            nc.sync.dma_start(out=csA_v[:, 1, :], in_=cs0raw[64:128, :])

            if stage == "A":
                bail()
                return nc

            # ---------------- pooled-map cascade (block sums) ----------------
            def pool_step(src_t, dst_t, h, w):
                """src [8, h*w] -> dst [8, (h/2)*(w/2)], 2x2 block sums."""
                w2, h2 = w // 2, h // 2
                tmp = sp.tile([8, h * w2], f32, tag="scratch8", name="pooltmp")
                nc.vector.reduce_sum(
                    tmp[:, :],
                    src_t.rearrange(
                        "p (h w2 two) -> p (h w2) two", h=h, w2=w2, two=2
                    ),
                    axis=mybir.AxisListType.X,
                )
                nc.vector.reduce_sum(
                    dst_t,
                    tmp.rearrange(
                        "p (h2 hp w2) -> p h2 w2 hp", h2=h2, hp=2, w2=w2
                    ),
                    axis=mybir.AxisListType.X,
                )

            for kind in ("s", "t"):
                for lv in range(4):
                    prev = mt[(kind, lv)][:, :]
                    for ridx in range(lv + 1, 4):
                        dst = pdt[(kind, lv, ridx)]
                        pool_step(prev, dst[:, :], RES[ridx - 1], RES[ridx - 1])
                        prev = dst[:, :]

            if stage == "B1":
                bail()
                return nc

            # ---------------- norms (1/L2) ----------------
            # needed: all natives; pooled student maps (for j finer than i)
            rn = {}

            def calc_rn(key_, ap, hw, tagn):
                prod = sp.tile([8, hw], f32, tag="scratch8", name="normprod")
                n2 = sp.tile([8, 1], f32, tag="n2", name="n2")
                nc.scalar.activation(prod[:, :], ap, AF.Square)
                nc.vector.reduce_sum(n2[:, :], prod[:, :], axis=mybir.AxisListType.X)
                sd = sp.tile([8, 1], f32, tag="normsd", name="normsd")
                nc.scalar.activation(sd[:, :], n2[:, :], AF.Sqrt)
                r = pp.tile([8, 1], f32, tag=f"rn_{tagn}", name=f"rn_{tagn}")
                nc.vector.reciprocal(r[:, :], sd[:, :])
                rn[key_] = r

            for kind in ("s", "t"):
                for lv in range(4):
                    calc_rn((kind, lv, lv), mt[(kind, lv)][:, :], HWS[lv],
                            f"{kind}{lv}n")
            for lv in range(4):
                for ridx in range(lv + 1, 4):
                    calc_rn(("s", lv, ridx), pdt[("s", lv, ridx)][:, :],
                            HWS[ridx], f"s{lv}p{ridx}")

            if stage == "B2":
                bail()
                return nc

            # ---------------- dots + diff ----------------
            # D[b, i*4+j] = mean_p (value_s[i][b,j] - value_t[i][b])^2
            D = pp.tile([BLOC, 16], f32, tag="D")
            for ti in range(4):
                for sj in range(4):
                    if sj <= ti:
                        # student at res sj is >= teacher res ti (finer or equal):
                        # pool student down to res ti (or native if equal)
                        u = (mt[("s", sj)] if sj == ti else pdt[("s", sj, ti)])[:, :]
                        v = mt[("t", ti)][:, :]
                        un = rn[("s", sj, ti)]
                        hwd = HWS[ti]
                        r2 = 1.0
                    else:
                        # student coarser: dot(S_j, blocksum(T_i -> res sj))
                        u = mt[("s", sj)][:, :]
                        v = pdt[("t", ti, sj)][:, :]
                        un = rn[("s", sj, sj)]
                        hwd = HWS[sj]
                        r2 = float(HWS[ti]) / HWS[sj]
                    vn = rn[("t", ti, ti)]
                    prod = sp.tile([8, hwd], f32, tag="scratch8", name="dotprod")
                    dot = sp.tile([8, 1], f32, tag="dot", name="dot")
                    nc.vector.tensor_mul(prod[:, :], u, v)
                    nc.vector.reduce_sum(dot[:, :], prod[:, :], axis=mybir.AxisListType.X)
                    nc.vector.tensor_mul(dot[:, :], dot[:, :], un[:, :])
                    nc.vector.tensor_mul(dot[:, :], dot[:, :], vn[:, :])
                    hw_i = HWS[ti]
                    a = -2.0 / (hw_i * float(np.sqrt(r2)))
                    col = ti * 4 + sj
                    nc.vector.tensor_scalar(
                        out=D[:, col : col + 1],
                        in0=dot[:, :],
                        scalar1=a,
                        scalar2=2.0 / hw_i,
                        op0=ALU.mult,
                        op1=ALU.add,
                    )

            if stage == "B":
                bail()
                return nc

            # ---------------- key/query matmuls + BN stats ----------------
            # weights to SBUF: [128, n_ct, 128]
            w_sb = {}
            for kind in ("s", "t"):
                for lv in range(4):
                    c = (S_SHAPES if kind == "s" else T_SHAPES)[lv][0]
                    wd = (wk if kind == "s" else wq)[lv]
                    if c >= 128:
                        nct = c // 128
                        t = pp.tile(
                            [128, nct, QK], f32,
                            tag=f"w_{kind}{lv}", name=f"w_{kind}{lv}",
                        )
                        nc.sync.dma_start(
                            out=t[:, :, :],
                            in_=wd.ap().rearrange("(ct p) q -> p ct q", p=128),
                        )
                    else:
                        t = pp.tile(
                            [64, 1, QK], f32,
                            tag=f"w_{kind}{lv}", name=f"w_{kind}{lv}",
                        )
                        nc.sync.dma_start(out=t[:, 0, :], in_=wd.ap())
                    w_sb[(kind, lv)] = t

            bT_sb = pp.tile([128, 8], f32, tag="bT_sb")
            nc.sync.dma_start(out=bT_sb[:, :], in_=bT8.ap())
            gT_sb = pp.tile([128, 8], f32, tag="gT_sb")
            nc.sync.dma_start(out=gT_sb[:, :], in_=gT8.ap())
            betaT_sb = pp.tile([128, 8], f32, tag="betaT_sb")
            nc.sync.dma_start(out=betaT_sb[:, :], in_=betaT8.ap())

            # layer order: 0-3 q (teacher levels), 4-7 k (student levels)
            LAYERS = [("t", i) for i in range(4)] + [("s", j) for j in range(4)]
            yT = pp.tile([128, 8, BLOC], f32, tag="yT")  # per layer [128, 8]
            ST = pp.tile([128, 8, 2], f32, tag="ST")
            for l, (kind, lv) in enumerate(LAYERS):
                w = w_sb[(kind, lv)]
                x = cs[(kind, lv)]
                kp = w.shape[0]  # 64 for s0 else 128
                nct = w.shape[1]
                py = pmm_pool.tile([128, BLOC], f32, tag="pmm")
                for ct in range(nct):
                    nc.tensor.matmul(
                        py[:, :],
                        w[0:kp, ct, :],
                        x[0:kp, ct, :],
                        start=(ct == 0),
                        stop=(ct == nct - 1),
                    )
                nc.scalar.activation(
                    yT[:, l, :], py[:, :], AF.Identity, bias=bT_sb[:, l : l + 1]
                )
                nc.vector.reduce_sum(
                    ST[:, l, 0:1], yT[:, l, :], axis=mybir.AxisListType.X
                )
                ysq = sp.tile([128, BLOC], f32, tag="ysq")
                nc.scalar.activation(ysq[:, :], yT[:, l, :], AF.Square)
                nc.vector.reduce_sum(
                    ST[:, l, 1:2], ysq[:, :], axis=mybir.AxisListType.X
                )

            # ---------------- AllReduce #1 (BN stats for q/k) ----------------
            cc1_in = dram_pool.tile([128, 16], f32)
            cc1_out = dram_pool.tile([128, 16], f32)
            nc.sync.dma_start(out=cc1_in[:, :], in_=ST[:, :, :])
            nc.gpsimd.collective_compute(
                "AllReduce",
                ALU.add,
                ins=[cc1_in.opt()],
                outs=[cc1_out.opt()],
                replica_groups=RG,
            )
            STg = pp.tile([128, 8, 2], f32, tag="STg")
            nc.sync.dma_start(out=STg[:, :, :], in_=cc1_out[:, :])

            # BN params vectorized over 8 layers
            def bn_params(STg_ap, nlayers, count, g_ap, beta_ap, tagp):
                m = sp.tile([128, nlayers], f32, tag=f"m{tagp}")
                nc.vector.tensor_scalar(
                    out=m[:, :], in0=STg_ap[:, :, 0], scalar1=1.0 / count,
                    scalar2=None, op0=ALU.mult,
                )
                ex2 = sp.tile([128, nlayers], f32, tag=f"ex2{tagp}")
                nc.vector.tensor_scalar(
                    out=ex2[:, :], in0=STg_ap[:, :, 1], scalar1=1.0 / count,
                    scalar2=None, op0=ALU.mult,
                )
                m2 = sp.tile([128, nlayers], f32, tag=f"m2{tagp}")
                nc.scalar.activation(m2[:, :], m[:, :], AF.Square)
                v = sp.tile([128, nlayers], f32, tag=f"v{tagp}")
                nc.vector.tensor_sub(v[:, :], ex2[:, :], m2[:, :])
                sd = sp.tile([128, nlayers], f32, tag=f"sd{tagp}")
                nc.scalar.activation(
                    sd[:, :], v[:, :], AF.Sqrt, bias=epsT[:, :]
                )
                rstd = sp.tile([128, nlayers], f32, tag=f"rstd{tagp}")
                nc.vector.reciprocal(rstd[:, :], sd[:, :])
                sc = pp.tile([128, nlayers], f32, tag=f"sc{tagp}")
                nc.vector.tensor_mul(sc[:, :], g_ap, rstd[:, :])
                msc = sp.tile([128, nlayers], f32, tag=f"msc{tagp}")
                nc.vector.tensor_mul(msc[:, :], m[:, :], sc[:, :])
                sh = pp.tile([128, nlayers], f32, tag=f"sh{tagp}")
                nc.vector.tensor_sub(sh[:, :], beta_ap, msc[:, :])
                return sc, sh

            sc8, sh8 = bn_params(STg, 8, float(BS), gT_sb[:, :], betaT_sb[:, :], "a")

            # apply: qn (layers 0-3), kn (layers 4-7, +relu)
            qn = pp.tile([128, 4, BLOC], f32, tag="qn")
            kn = pp.tile([128, 4, BLOC], f32, tag="kn")
            for l in range(8):
                dst = qn[:, l, :] if l < 4 else kn[:, l - 4, :]
                nc.vector.tensor_scalar(
                    out=dst,
                    in0=yT[:, l, :],
                    scalar1=sc8[:, l : l + 1],
                    scalar2=sh8[:, l : l + 1],
                    op0=ALU.mult,
                    op1=ALU.add,
                )
            for j in range(4):
                nc.scalar.activation(kn[:, j, :], kn[:, j, :], AF.Relu)

            # ---------------- bilinear ----------------
            wb_sb = pp.tile([128, 512], f32, tag="wb_sb")
            nc.sync.dma_start(out=wb_sb[:, :], in_=wb.ap())
            bbT_sb = pp.tile([128, 4], f32, tag="bbT_sb")
            nc.sync.dma_start(out=bbT_sb[:, :], in_=bbT.ap())
            gbT_sb = pp.tile([128, 4], f32, tag="gbT_sb")
            nc.sync.dma_start(out=gbT_sb[:, :], in_=gbT.ap())
            betabT_sb = pp.tile([128, 4], f32, tag="betabT_sb")
            nc.sync.dma_start(out=betabT_sb[:, :], in_=betabT.ap())

            # K32[:, b*4+j] = kn[:, j, b]
            K32 = pp.tile([128, 32], f32, tag="K32")
            for j in range(4):
                nc.scalar.activation(
                    K32.rearrange("p (b j) -> p b j", j=4)[:, :, j],
                    kn[:, j, :],
                    AF.Copy,
                )

            kbT = pp.tile([128, 4, 32], f32, tag="kbT")  # per t-block [128, 32]
            STb = pp.tile([128, 4, 2], f32, tag="STb")
            for m in range(4):
                pkb = pmm_pool.tile([128, 32], f32, tag="pmm")
                nc.tensor.matmul(
                    pkb[:, :],
                    wb_sb[:, m * 128 : (m + 1) * 128],
                    K32[:, :],
                    start=True,
                    stop=True,
                )
                nc.scalar.activation(
                    kbT[:, m, :], pkb[:, :], AF.Identity, bias=bbT_sb[:, m : m + 1]
                )
                nc.vector.reduce_sum(
                    STb[:, m, 0:1], kbT[:, m, :], axis=mybir.AxisListType.X
                )
                kbsq = sp.tile([128, 32], f32, tag="kbsq")
                nc.scalar.activation(kbsq[:, :], kbT[:, m, :], AF.Square)
                nc.vector.reduce_sum(
                    STb[:, m, 1:2], kbsq[:, :], axis=mybir.AxisListType.X
                )

            if stage == "C":
                bail()
                return nc

            # ---------------- AllReduce #2 (bilinear BN stats) ----------------
            cc2_in = dram_pool.tile([128, 8], f32)
            cc2_out = dram_pool.tile([128, 8], f32)
            nc.sync.dma_start(out=cc2_in[:, :], in_=STb[:, :, :])
            nc.gpsimd.collective_compute(
                "AllReduce",
                ALU.add,
                ins=[cc2_in.opt()],
                outs=[cc2_out.opt()],
                replica_groups=RG,
            )
            STbg = pp.tile([128, 4, 2], f32, tag="STbg")
            nc.sync.dma_start(out=STbg[:, :, :], in_=cc2_out[:, :])

            scb, shb = bn_params(
                STbg, 4, float(BS * 4), gbT_sb[:, :], betabT_sb[:, :], "b"
            )
            kbn = pp.tile([128, 4, 32], f32, tag="kbn")
            for m in range(4):
                nc.vector.tensor_scalar(
                    out=kbn[:, m, :],
                    in0=kbT[:, m, :],
                    scalar1=scb[:, m : m + 1],
                    scalar2=shb[:, m : m + 1],
                    op0=ALU.mult,
                    op1=ALU.add,
                )

            # ---------------- logits ----------------
            # full cross matmul per teacher i: out[b', (b,j)] then mask+reduce
            LTl = pp.tile([BLOC, 16], f32, tag="LTl")
            for i in range(4):
                pl = pmm_pool.tile([BLOC, 32], f32, tag="pmm")
                nc.tensor.matmul(
                    pl[:, :], qn[:, i, :], kbn[:, i, :], start=True, stop=True
                )
                ml = sp.tile([BLOC, 32], f32, tag="ml")
                nc.vector.tensor_mul(ml[:, :], pl[:, :], mask_sb[:, :])
                nc.vector.reduce_sum(
                    LTl[:, i * 4 : (i + 1) * 4],
                    ml.rearrange("p (b j) -> p j b", j=4),
                    axis=mybir.AxisListType.X,
                )

            if stage == "D":
                bail()
                return nc

            # ---------------- softmax + loss ----------------
            mx = sp.tile([BLOC, 4], f32, tag="mx")
            nc.vector.reduce_max(
                mx[:, :],
                LTl.rearrange("p (i j) -> p i j", j=4),
                axis=mybir.AxisListType.X,
            )
            LS = sp.tile([BLOC, 16], f32, tag="LS")
            for j in range(4):
                nc.vector.tensor_sub(
                    LS.rearrange("p (i j) -> p i j", j=4)[:, :, j],
                    LTl.rearrange("p (i j) -> p i j", j=4)[:, :, j],
                    mx[:, :],
                )
            E = sp.tile([BLOC, 16], f32, tag="E")
            nc.scalar.activation(E[:, :], LS[:, :], AF.Exp)
            Z = sp.tile([BLOC, 4], f32, tag="Z")
            nc.vector.reduce_sum(
                Z[:, :], E.rearrange("p (i j) -> p i j", j=4),
                axis=mybir.AxisListType.X,
            )
            ED = sp.tile([BLOC, 16], f32, tag="ED")
            nc.vector.tensor_mul(ED[:, :], E[:, :], D[:, :])
            NUM = sp.tile([BLOC, 4], f32, tag="NUM")
            nc.vector.reduce_sum(
                NUM[:, :], ED.rearrange("p (i j) -> p i j", j=4),
                axis=mybir.AxisListType.X,
            )
            Zi = sp.tile([BLOC, 4], f32, tag="Zi")
            nc.vector.reciprocal(Zi[:, :], Z[:, :])
            R8 = sp.tile([BLOC, 4], f32, tag="R8")
            nc.vector.tensor_mul(R8[:, :], NUM[:, :], Zi[:, :])
            ploss = pmm_pool.tile([1, 4], f32, tag="pmm")
            nc.tensor.matmul(
                ploss[:, :], ones128[0:BLOC, :], R8[:, :], start=True, stop=True
            )
            lossloc = sp.tile([1, 4], f32, tag="lossloc")
            nc.scalar.activation(
                lossloc[:, :], ploss[:, :], AF.Copy, scale=1.0 / BS
            )

            # ---------------- AllReduce #3 (loss mean) ----------------
            cc3_in = dram_pool.tile([1, 4], f32)
            cc3_out = dram_pool.tile([1, 4], f32)
            nc.sync.dma_start(out=cc3_in[:, :], in_=lossloc[:, :])
            nc.gpsimd.collective_compute(
                "AllReduce",
                ALU.add,
                ins=[cc3_in.opt()],
                outs=[cc3_out.opt()],
                replica_groups=RG,
            )
            nc.sync.dma_start(out=out_ext[:, :], in_=cc3_out[:, :])

    return nc


_CACHE = {}


def _build_and_finalize():
    if "nc" in _CACHE:
        return _CACHE["nc"]
    import concourse.bacc as bacc

    nc = bacc.Bacc("TRN2", target_bir_lowering=False, debug=False, num_devices=NCORES)
    build(nc)
    nc.finalize()
    _CACHE["nc"] = nc
    return nc


def _host_prep(inputs):
    """Build the 8 per-core input maps from full inputs."""
    a = {k: np.ascontiguousarray(np.asarray(v, dtype=np.float32)) for k, v in inputs.items()}
    shared = {}
    for j in range(4):
        shared[f"wk{j}"] = np.ascontiguousarray(a[f"Wk{j}"] / HWS[j])
    for i in range(4):
        shared[f"wq{i}"] = np.ascontiguousarray(a[f"Wq{i}"] / HWS[i])
    shared["wb"] = a["Wb"]
    # layers 0-3 = q, 4-7 = k; transposed to [128, 8]
    shared["bT8"] = np.ascontiguousarray(
        np.concatenate([a["bq"].T, a["bk"].T], axis=1)
    )
    shared["gT8"] = np.ascontiguousarray(
        np.concatenate([a["gq"].T, a["gk"].T], axis=1)
    )
    shared["betaT8"] = np.ascontiguousarray(
        np.concatenate([a["betaq"].T, a["betak"].T], axis=1)
    )
    shared["bbT"] = np.ascontiguousarray(a["bb"].reshape(4, 128).T)
    shared["gbT"] = np.ascontiguousarray(a["gb"].reshape(4, 128).T)
    shared["betabT"] = np.ascontiguousarray(a["betab"].reshape(4, 128).T)
    mask = np.zeros((BLOC, BLOC * 4), dtype=np.float32)
    for b in range(BLOC):
        mask[b, b * 4 : (b + 1) * 4] = 1.0
    shared["mask8"] = mask

    in_maps = []
    for cidx in range(NCORES):
        m = dict(shared)
        sl = slice(cidx * BLOC, (cidx + 1) * BLOC)
        for j in range(4):
            m[f"g_s{j}"] = np.ascontiguousarray(a[f"g_s{j}"][sl])
        for i in range(4):
            m[f"g_t{i}"] = np.ascontiguousarray(a[f"g_t{i}"][sl])
        in_maps.append(m)
    return in_maps


def run_on_hw(inputs, trace=False):
    from concourse.bass_utils import run_bass_kernel_spmd

    nc = _build_and_finalize()
    in_maps = _host_prep(inputs)
    res = run_bass_kernel_spmd(
        nc, in_maps, core_ids=list(range(NCORES)), trace=trace
    )
    return res


def kernel(**inputs):
    res = run_on_hw(inputs, trace=False)
    return np.asarray(res.results[0]["out"], dtype=np.float32).reshape(4)
